# revision 2
# baseline (speedup 1.0000x reference)
"""Bilinear RoI pooling (7x7) on 8 Trainium2 NeuronCores.

Strategy (data-parallel over RoIs, per the sharding hint):
  - B=1024 boxes split into 8 slices of 128; the feature map is replicated.
  - Host pads features (128,128,512) -> (130,132,512) with a zero border
    (1 row/col top/left, 2 pad cols left+right) and corner indices are
    clamped so every out-of-bounds bilinear corner reads a zero row instead
    of needing an in-bounds mask.
  - Per core, per (box, grid-point): indirect-DMA gathers of 1024
    contiguous f32 (the x0/x0+1 row pair for each of the two y corners),
    then a 4-term per-partition weighted sum on the vector engine.
  - Gathers are grouped: one indirect DMA fetches gk grid points' worth of
    A- and B-rows (2*gk indices/partition) to amortize SWDGE overhead.

Device layout: partition = box (128/core); 49 grid points along free dim.
"""

import numpy as np

P = 128          # boxes per core == SBUF partitions
C = 512          # channels
NPT = 49         # 7*7 grid points
HP, WP = 130, 132
NROW = HP * WP   # 17160 padded rows of C floats
NCORES = 8
MAGIC = 12582912.0  # 1.5*2^23: x+MAGIC stays in [2^23,2^24) where ulp == 1

_STATE = {}


# NOTE: multi-index offset APs (merge_ab / gk>1) pass CoreSim but produce
# garbage on real hardware — the HW indirect DMA only honors [P,1] offsets.
def _build_nc(repeats=1, skip_b=False, skip_blend=False, bufs=16, gk=1,
              gbufs=2, abufs=3, merge_ab=False, act_offload=False,
              store_group=7):
    import concourse.bass as bass
    import concourse.bacc as bacc
    import concourse.tile as tile
    from concourse import mybir

    F32 = mybir.dt.float32
    I32 = mybir.dt.int32
    Alu = mybir.AluOpType

    nc = bacc.Bacc()
    fpad = nc.declare_dram_parameter("fpad", [NROW, C], F32, isOutput=False)
    boxes = nc.declare_dram_parameter("boxes", [P, 4], F32, isOutput=False)
    grid = nc.declare_dram_parameter("grid", [P, 2 * NPT], F32, isOutput=False)
    out = nc.declare_dram_parameter("out", [P, NPT * C], F32, isOutput=True)

    with tile.TileContext(nc) as tc:
        with (
            tc.tile_pool(name="const", bufs=1) as cpool,
            tc.tile_pool(name="gpool", bufs=gbufs) as gpool,
            tc.tile_pool(name="apool", bufs=abufs) as apool,
            tc.tile_pool(name="work", bufs=bufs) as wpool,
        ):
            bx = cpool.tile([P, 4], F32)
            nc.sync.dma_start(out=bx[:], in_=boxes[:])
            g = cpool.tile([P, 2 * NPT], F32)
            nc.sync.dma_start(out=g[:], in_=grid[:])
            BY = g[:, 0:NPT]
            BX = g[:, NPT:2 * NPT]

            xc, yc = bx[:, 0:1], bx[:, 1:2]
            bw, bh = bx[:, 2:3], bx[:, 3:4]

            # per-box scale/translate: yf = BY*(0.5*bh-0.5) + (yc-1)
            sy = cpool.tile([P, 1], F32)
            nc.vector.tensor_scalar(out=sy[:], in0=bh, scalar1=0.5, scalar2=-0.5,
                                    op0=Alu.mult, op1=Alu.add)
            sx = cpool.tile([P, 1], F32)
            nc.vector.tensor_scalar(out=sx[:], in0=bw, scalar1=0.5, scalar2=-0.5,
                                    op0=Alu.mult, op1=Alu.add)
            ty = cpool.tile([P, 1], F32)
            nc.vector.tensor_scalar(out=ty[:], in0=yc, scalar1=-1.0, scalar2=None,
                                    op0=Alu.add)
            tx = cpool.tile([P, 1], F32)
            nc.vector.tensor_scalar(out=tx[:], in0=xc, scalar1=-1.0, scalar2=None,
                                    op0=Alu.add)

            yf = cpool.tile([P, NPT], F32)
            nc.vector.tensor_scalar(out=yf[:], in0=BY, scalar1=sy[:, 0:1],
                                    scalar2=ty[:, 0:1], op0=Alu.mult, op1=Alu.add)
            xf = cpool.tile([P, NPT], F32)
            nc.vector.tensor_scalar(out=xf[:], in0=BX, scalar1=sx[:, 0:1],
                                    scalar2=tx[:, 0:1], op0=Alu.mult, op1=Alu.add)

            def floor_frac(src, nm):
                r = cpool.tile([P, NPT], F32, tag=f"r{nm}")
                nc.vector.tensor_scalar(out=r[:], in0=src[:], scalar1=MAGIC,
                                        scalar2=-MAGIC, op0=Alu.add, op1=Alu.add)
                m = cpool.tile([P, NPT], F32, tag=f"m{nm}")
                nc.vector.tensor_tensor(out=m[:], in0=r[:], in1=src[:], op=Alu.is_gt)
                fl = cpool.tile([P, NPT], F32, tag=f"f{nm}")
                nc.vector.tensor_tensor(out=fl[:], in0=r[:], in1=m[:], op=Alu.subtract)
                fr = cpool.tile([P, NPT], F32, tag=f"w{nm}")
                nc.vector.tensor_tensor(out=fr[:], in0=src[:], in1=fl[:], op=Alu.subtract)
                return fl, fr

            y0, wy = floor_frac(yf[:], "y")
            x0, wx = floor_frac(xf[:], "x")

            wyc = cpool.tile([P, NPT], F32)
            nc.vector.tensor_scalar(out=wyc[:], in0=wy[:], scalar1=-1.0, scalar2=1.0,
                                    op0=Alu.mult, op1=Alu.add)
            wxc = cpool.tile([P, NPT], F32)
            nc.vector.tensor_scalar(out=wxc[:], in0=wx[:], scalar1=-1.0, scalar2=1.0,
                                    op0=Alu.mult, op1=Alu.add)

            wA0 = cpool.tile([P, NPT], F32)
            nc.vector.tensor_tensor(out=wA0[:], in0=wyc[:], in1=wxc[:], op=Alu.mult)
            wA1 = cpool.tile([P, NPT], F32)
            nc.vector.tensor_tensor(out=wA1[:], in0=wyc[:], in1=wx[:], op=Alu.mult)
            wB0 = cpool.tile([P, NPT], F32)
            nc.vector.tensor_tensor(out=wB0[:], in0=wy[:], in1=wxc[:], op=Alu.mult)
            wB1 = cpool.tile([P, NPT], F32)
            nc.vector.tensor_tensor(out=wB1[:], in0=wy[:], in1=wx[:], op=Alu.mult)

            # gather row indices into the padded (130,132) map, in 512-elem
            # units:
            #   idxA = (clamp(y0,-1,128)+1)*132 + clamp(x0,-2,128)+2
            #   idxB = (clamp(y0,-2,127)+2)*132 + clamp(x0,-2,128)+2
            cy0 = cpool.tile([P, NPT], F32)
            nc.vector.tensor_scalar(out=cy0[:], in0=y0[:], scalar1=-1.0, scalar2=128.0,
                                    op0=Alu.max, op1=Alu.min)
            cy1 = cpool.tile([P, NPT], F32)
            nc.vector.tensor_scalar(out=cy1[:], in0=y0[:], scalar1=127.0, scalar2=-2.0,
                                    op0=Alu.min, op1=Alu.max)
            cxc = cpool.tile([P, NPT], F32)
            nc.vector.tensor_scalar(out=cxc[:], in0=x0[:], scalar1=-2.0, scalar2=128.0,
                                    op0=Alu.max, op1=Alu.min)

            affA = cpool.tile([P, NPT], F32)
            nc.vector.tensor_scalar(out=affA[:], in0=cy0[:], scalar1=float(WP),
                                    scalar2=float(WP + 2), op0=Alu.mult, op1=Alu.add)
            nc.vector.tensor_tensor(out=affA[:], in0=affA[:], in1=cxc[:], op=Alu.add)
            affB = cpool.tile([P, NPT], F32)
            nc.vector.tensor_scalar(out=affB[:], in0=cy1[:], scalar1=float(WP),
                                    scalar2=float(2 * WP + 2), op0=Alu.mult, op1=Alu.add)
            nc.vector.tensor_tensor(out=affB[:], in0=affB[:], in1=cxc[:], op=Alu.add)

            # NOTE: do NOT try to skip pad-zero gathers via
            # bounds_check+oob_is_err=False — a skipped descriptor leaves the
            # dest SBUF unwritten, and 0 * stale-NaN = NaN when uninitialized
            # SBUF holds NaN bit patterns (observed nondeterministically on
            # HW). The zero-padded feature map already makes out-of-bounds
            # corners contribute exactly 0.

            import concourse.bass as _b

            if gk > 1:
                # grouped gathers: one indirect DMA per gk grid points, with
                # A indices then B indices per group:
                #   idxAB cols [g*2gk : g*2gk+gk]       = idxA[t0 : t0+gk]
                #   idxAB cols [g*2gk+gk : (g+1)*2gk]   = idxB[t0 : t0+gk]
                ng = NPT // gk
                assert NPT % gk == 0
                idxAB = cpool.tile([P, 2 * NPT], I32)
                idxAB4 = idxAB[:].rearrange("p (g two k) -> p g two k",
                                            g=ng, two=2, k=gk)
                nc.vector.tensor_copy(
                    out=idxAB4[:, :, 0, :],
                    in_=affA[:].rearrange("p (g k) -> p g k", g=ng, k=gk))
                nc.vector.tensor_copy(
                    out=idxAB4[:, :, 1, :],
                    in_=affB[:].rearrange("p (g k) -> p g k", g=ng, k=gk))
                for rep in range(repeats):
                    for g_i in range(ng):
                        gfat = gpool.tile([P, gk * 4 * C], F32, tag="gfat")
                        nc.gpsimd.indirect_dma_start(
                            out=gfat[:], out_offset=None, in_=fpad[:],
                            in_offset=_b.IndirectOffsetOnAxis(
                                ap=idxAB[:, g_i * 2 * gk:(g_i + 1) * 2 * gk],
                                axis=0),
                        )
                        afat = apool.tile([P, gk * C], F32, tag="afat")
                        for k in range(gk):
                            t = g_i * gk + k
                            a0 = gfat[:, k * 2 * C: k * 2 * C + C]
                            a1 = gfat[:, k * 2 * C + C: (k + 1) * 2 * C]
                            b0 = gfat[:, (gk + k) * 2 * C: (gk + k) * 2 * C + C]
                            b1 = gfat[:, (gk + k) * 2 * C + C: (gk + k + 1) * 2 * C]
                            ac = afat[:, k * C:(k + 1) * C]
                            nc.vector.tensor_scalar(
                                out=ac, in0=a0, scalar1=wA0[:, t:t + 1],
                                scalar2=None, op0=Alu.mult)
                            nc.vector.scalar_tensor_tensor(
                                out=ac, in0=a1, scalar=wA1[:, t:t + 1], in1=ac,
                                op0=Alu.mult, op1=Alu.add)
                            nc.vector.scalar_tensor_tensor(
                                out=ac, in0=b0, scalar=wB0[:, t:t + 1], in1=ac,
                                op0=Alu.mult, op1=Alu.add)
                            nc.vector.scalar_tensor_tensor(
                                out=ac, in0=b1, scalar=wB1[:, t:t + 1], in1=ac,
                                op0=Alu.mult, op1=Alu.add)
                        nc.sync.dma_start(
                            out=out[:, g_i * gk * C:(g_i + 1) * gk * C],
                            in_=afat[:])
            elif merge_ab:
                # one gather per point with both y-corner indices:
                # idxAB2 cols [2t, 2t+1] = idxA[t], idxB[t]
                idxAB2 = cpool.tile([P, 2 * NPT], I32)
                iv = idxAB2[:].rearrange("p (t two) -> p t two", t=NPT, two=2)
                nc.vector.tensor_copy(out=iv[:, :, 0], in_=affA[:])
                nc.vector.tensor_copy(out=iv[:, :, 1], in_=affB[:])
                for t in [t for _ in range(repeats) for t in range(NPT)]:
                    gAB = wpool.tile([P, 4 * C], F32, tag="gAB")
                    nc.gpsimd.indirect_dma_start(
                        out=gAB[:], out_offset=None, in_=fpad[:],
                        in_offset=_b.IndirectOffsetOnAxis(
                            ap=idxAB2[:, 2 * t:2 * t + 2], axis=0),
                    )
                    acc = wpool.tile([P, C], F32, tag="acc")
                    if act_offload:
                        import concourse.mybir as _mb
                        m = wpool.tile([P, C], F32, tag="actm")
                        nc.scalar.activation(out=m[:], in_=gAB[:, 2 * C:3 * C],
                                             func=_mb.ActivationFunctionType.Copy,
                                             scale=wB0[:, t:t + 1])
                        nc.vector.scalar_tensor_tensor(out=acc[:],
                                                       in0=gAB[:, 0:C],
                                                       scalar=wA0[:, t:t + 1],
                                                       in1=m[:],
                                                       op0=Alu.mult, op1=Alu.add)
                    else:
                        nc.vector.tensor_scalar(out=acc[:], in0=gAB[:, 0:C],
                                                scalar1=wA0[:, t:t + 1],
                                                scalar2=None, op0=Alu.mult)
                    nc.vector.scalar_tensor_tensor(out=acc[:], in0=gAB[:, C:2 * C],
                                                   scalar=wA1[:, t:t + 1],
                                                   in1=acc[:],
                                                   op0=Alu.mult, op1=Alu.add)
                    if not act_offload:
                        nc.vector.scalar_tensor_tensor(out=acc[:],
                                                       in0=gAB[:, 2 * C:3 * C],
                                                       scalar=wB0[:, t:t + 1],
                                                       in1=acc[:],
                                                       op0=Alu.mult, op1=Alu.add)
                    nc.vector.scalar_tensor_tensor(out=acc[:],
                                                   in0=gAB[:, 3 * C:4 * C],
                                                   scalar=wB1[:, t:t + 1],
                                                   in1=acc[:],
                                                   op0=Alu.mult, op1=Alu.add)
                    nc.sync.dma_start(out=out[:, t * C:(t + 1) * C], in_=acc[:])
            elif store_group > 1:
                # same per-point gathers/blends, but blends write into a
                # [P, store_group*C] tile flushed by one contiguous store
                # per group (fewer, larger store descriptors)
                sg = store_group
                assert NPT % sg == 0
                idxA = cpool.tile([P, NPT], I32)
                nc.vector.tensor_copy(out=idxA[:], in_=affA[:])
                idxB = cpool.tile([P, NPT], I32)
                nc.vector.tensor_copy(out=idxB[:], in_=affB[:])
                for rep in range(repeats):
                    for g_i in range(NPT // sg):
                        afat = apool.tile([P, sg * C], F32, tag="afat")
                        for k in range(sg):
                            t = g_i * sg + k
                            gA = wpool.tile([P, 2 * C], F32, tag="gA")
                            nc.gpsimd.indirect_dma_start(
                                out=gA[:], out_offset=None, in_=fpad[:],
                                in_offset=_b.IndirectOffsetOnAxis(
                                    ap=idxA[:, t:t + 1], axis=0))
                            gB = wpool.tile([P, 2 * C], F32, tag="gB")
                            nc.gpsimd.indirect_dma_start(
                                out=gB[:], out_offset=None, in_=fpad[:],
                                in_offset=_b.IndirectOffsetOnAxis(
                                    ap=idxB[:, t:t + 1], axis=0))
                            ac = afat[:, k * C:(k + 1) * C]
                            nc.vector.tensor_scalar(
                                out=ac, in0=gA[:, 0:C], scalar1=wA0[:, t:t + 1],
                                scalar2=None, op0=Alu.mult)
                            nc.vector.scalar_tensor_tensor(
                                out=ac, in0=gA[:, C:2 * C],
                                scalar=wA1[:, t:t + 1], in1=ac,
                                op0=Alu.mult, op1=Alu.add)
                            nc.vector.scalar_tensor_tensor(
                                out=ac, in0=gB[:, 0:C],
                                scalar=wB0[:, t:t + 1], in1=ac,
                                op0=Alu.mult, op1=Alu.add)
                            nc.vector.scalar_tensor_tensor(
                                out=ac, in0=gB[:, C:2 * C],
                                scalar=wB1[:, t:t + 1], in1=ac,
                                op0=Alu.mult, op1=Alu.add)
                        nc.sync.dma_start(
                            out=out[:, g_i * sg * C:(g_i + 1) * sg * C],
                            in_=afat[:])
            else:
                idxA = cpool.tile([P, NPT], I32)
                nc.vector.tensor_copy(out=idxA[:], in_=affA[:])
                idxB = cpool.tile([P, NPT], I32)
                nc.vector.tensor_copy(out=idxB[:], in_=affB[:])
                for t in [t for _ in range(repeats) for t in range(NPT)]:
                    gA = wpool.tile([P, 2 * C], F32, tag="gA")
                    nc.gpsimd.indirect_dma_start(
                        out=gA[:], out_offset=None, in_=fpad[:],
                        in_offset=_b.IndirectOffsetOnAxis(ap=idxA[:, t:t + 1],
                                                          axis=0),
                    )
                    if not skip_b:
                        gB = wpool.tile([P, 2 * C], F32, tag="gB")
                        nc.gpsimd.indirect_dma_start(
                            out=gB[:], out_offset=None, in_=fpad[:],
                            in_offset=_b.IndirectOffsetOnAxis(ap=idxB[:, t:t + 1],
                                                              axis=0),
                        )
                    if skip_blend:
                        nc.sync.dma_start(out=out[:, t * C:(t + 1) * C],
                                          in_=gA[:, 0:C])
                        continue
                    acc = wpool.tile([P, C], F32, tag="acc")
                    nc.vector.tensor_scalar(out=acc[:], in0=gA[:, 0:C],
                                            scalar1=wA0[:, t:t + 1], scalar2=None,
                                            op0=Alu.mult)
                    nc.vector.scalar_tensor_tensor(out=acc[:], in0=gA[:, C:2 * C],
                                                   scalar=wA1[:, t:t + 1],
                                                   in1=acc[:],
                                                   op0=Alu.mult, op1=Alu.add)
                    if not skip_b:
                        nc.vector.scalar_tensor_tensor(out=acc[:],
                                                       in0=gB[:, 0:C],
                                                       scalar=wB0[:, t:t + 1],
                                                       in1=acc[:],
                                                       op0=Alu.mult, op1=Alu.add)
                        nc.vector.scalar_tensor_tensor(out=acc[:],
                                                       in0=gB[:, C:2 * C],
                                                       scalar=wB1[:, t:t + 1],
                                                       in1=acc[:],
                                                       op0=Alu.mult, op1=Alu.add)
                    nc.sync.dma_start(out=out[:, t * C:(t + 1) * C], in_=acc[:])

    nc.compile()
    return nc


def _grid_const():
    base = np.linspace(-1.0, 1.0, 7).astype(np.float32)
    by = np.repeat(base, 7)
    bxx = np.tile(base, 7)
    g = np.concatenate([by, bxx])[None, :]
    return np.ascontiguousarray(np.broadcast_to(g, (P, 2 * NPT)).astype(np.float32))


def _pad_features(features):
    fp = np.zeros((HP, WP, C), dtype=np.float32)
    fp[1:129, 2:130, :] = features
    return fp.reshape(NROW, C)


def _core_inputs(fpad, boxes, k):
    if "grid" not in _STATE:
        _STATE["grid"] = _grid_const()
    return {
        "fpad": fpad,
        "boxes": np.ascontiguousarray(boxes[k * P:(k + 1) * P]),
        "grid": _STATE["grid"],
    }


def kernel(features, boxes, image_height=128, image_width=128):
    from concourse.bass_utils import run_bass_kernel_spmd

    if "nc" not in _STATE:
        _STATE["nc"] = _build_nc()
    nc = _STATE["nc"]

    fpad = _pad_features(np.asarray(features, dtype=np.float32))
    boxes = np.asarray(boxes, dtype=np.float32)
    in_maps = [_core_inputs(fpad, boxes, k) for k in range(NCORES)]
    res = run_bass_kernel_spmd(
        nc, in_maps, core_ids=list(range(NCORES)),
        trace=_STATE.get("trace", False),
    )
    _STATE["last"] = res
    out = np.concatenate(
        [res.results[k]["out"].reshape(P, 7, 7, C) for k in range(NCORES)], axis=0
    )
    return out



# revision 3
# speedup vs baseline: 1.6193x; 1.6193x over previous
"""Bilinear RoI pooling (7x7) on 8 Trainium2 NeuronCores.

Strategy (data-parallel over RoIs, per the sharding hint):
  - B=1024 boxes split into 8 slices of 128; the feature map is replicated.
  - Host builds a bf16 "quad layout" map: Q[iy, ix] = the 2x2 corner block
    [F[y,x], F[y,x+1], F[y+1,x], F[y+1,x+1]] stored contiguously (4*C values),
    over a zero-padded canvas (2 pad rows/cols on every side).  One indirect
    DMA descriptor per (box, grid-point) fetches all 4 bilinear corners.
  - Corner indices are clamped to [-2, 128] so every out-of-bounds corner
    lands in an all-zero quad (the reference zeroes OOB contributions).
  - Blend: 4-term per-partition weighted sum on the vector engine with f32
    accumulator, final op writes bf16; stores are grouped and the host
    upcasts bf16 -> f32.

Device layout: partition = box (128/core); 49 grid points along free dim.
"""

import numpy as np
import ml_dtypes

P = 128          # boxes per core == SBUF partitions
C = 512          # channels
NPT = 49         # 7*7 grid points
HP = WP = 132    # padded canvas (2 zero rows/cols each side)
NQ = 131         # quad map is NQ x NQ cells of 4*C values
NROW = NQ * NQ
NCORES = 8
MAGIC = 12582912.0  # 1.5*2^23: x+MAGIC stays in [2^23,2^24) where ulp == 1

_STATE = {}


# NOTE: multi-index offset APs pass CoreSim but produce garbage on real
# hardware — the HW indirect DMA only honors [P,1] offsets.
def _build_nc(repeats=1, bufs=8, abufs=3, store_group=7):
    import concourse.bass as bass
    import concourse.bacc as bacc
    import concourse.tile as tile
    from concourse import mybir

    F32 = mybir.dt.float32
    BF16 = mybir.dt.bfloat16
    I32 = mybir.dt.int32
    Alu = mybir.AluOpType

    nc = bacc.Bacc()
    qmap = nc.declare_dram_parameter("qmap", [NROW, 4 * C], BF16, isOutput=False)
    boxes = nc.declare_dram_parameter("boxes", [P, 4], F32, isOutput=False)
    grid = nc.declare_dram_parameter("grid", [P, 2 * NPT], F32, isOutput=False)
    out = nc.declare_dram_parameter("out", [P, NPT * C], BF16, isOutput=True)

    with tile.TileContext(nc) as tc:
        with (
            tc.tile_pool(name="const", bufs=1) as cpool,
            tc.tile_pool(name="apool", bufs=abufs) as apool,
            tc.tile_pool(name="work", bufs=bufs) as wpool,
        ):
            bx = cpool.tile([P, 4], F32)
            nc.sync.dma_start(out=bx[:], in_=boxes[:])
            g = cpool.tile([P, 2 * NPT], F32)
            nc.sync.dma_start(out=g[:], in_=grid[:])
            BY = g[:, 0:NPT]
            BX = g[:, NPT:2 * NPT]

            xc, yc = bx[:, 0:1], bx[:, 1:2]
            bw, bh = bx[:, 2:3], bx[:, 3:4]

            # per-box scale/translate: yf = BY*(0.5*bh-0.5) + (yc-1)
            sy = cpool.tile([P, 1], F32)
            nc.vector.tensor_scalar(out=sy[:], in0=bh, scalar1=0.5, scalar2=-0.5,
                                    op0=Alu.mult, op1=Alu.add)
            sx = cpool.tile([P, 1], F32)
            nc.vector.tensor_scalar(out=sx[:], in0=bw, scalar1=0.5, scalar2=-0.5,
                                    op0=Alu.mult, op1=Alu.add)
            ty = cpool.tile([P, 1], F32)
            nc.vector.tensor_scalar(out=ty[:], in0=yc, scalar1=-1.0, scalar2=None,
                                    op0=Alu.add)
            tx = cpool.tile([P, 1], F32)
            nc.vector.tensor_scalar(out=tx[:], in0=xc, scalar1=-1.0, scalar2=None,
                                    op0=Alu.add)

            yf = cpool.tile([P, NPT], F32)
            nc.vector.tensor_scalar(out=yf[:], in0=BY, scalar1=sy[:, 0:1],
                                    scalar2=ty[:, 0:1], op0=Alu.mult, op1=Alu.add)
            xf = cpool.tile([P, NPT], F32)
            nc.vector.tensor_scalar(out=xf[:], in0=BX, scalar1=sx[:, 0:1],
                                    scalar2=tx[:, 0:1], op0=Alu.mult, op1=Alu.add)

            def floor_frac(src, nm):
                r = cpool.tile([P, NPT], F32, tag=f"r{nm}")
                nc.vector.tensor_scalar(out=r[:], in0=src[:], scalar1=MAGIC,
                                        scalar2=-MAGIC, op0=Alu.add, op1=Alu.add)
                m = cpool.tile([P, NPT], F32, tag=f"m{nm}")
                nc.vector.tensor_tensor(out=m[:], in0=r[:], in1=src[:], op=Alu.is_gt)
                fl = cpool.tile([P, NPT], F32, tag=f"f{nm}")
                nc.vector.tensor_tensor(out=fl[:], in0=r[:], in1=m[:], op=Alu.subtract)
                fr = cpool.tile([P, NPT], F32, tag=f"w{nm}")
                nc.vector.tensor_tensor(out=fr[:], in0=src[:], in1=fl[:], op=Alu.subtract)
                return fl, fr

            y0, wy = floor_frac(yf[:], "y")
            x0, wx = floor_frac(xf[:], "x")

            wyc = cpool.tile([P, NPT], F32)
            nc.vector.tensor_scalar(out=wyc[:], in0=wy[:], scalar1=-1.0, scalar2=1.0,
                                    op0=Alu.mult, op1=Alu.add)
            wxc = cpool.tile([P, NPT], F32)
            nc.vector.tensor_scalar(out=wxc[:], in0=wx[:], scalar1=-1.0, scalar2=1.0,
                                    op0=Alu.mult, op1=Alu.add)

            wA0 = cpool.tile([P, NPT], F32)
            nc.vector.tensor_tensor(out=wA0[:], in0=wyc[:], in1=wxc[:], op=Alu.mult)
            wA1 = cpool.tile([P, NPT], F32)
            nc.vector.tensor_tensor(out=wA1[:], in0=wyc[:], in1=wx[:], op=Alu.mult)
            wB0 = cpool.tile([P, NPT], F32)
            nc.vector.tensor_tensor(out=wB0[:], in0=wy[:], in1=wxc[:], op=Alu.mult)
            wB1 = cpool.tile([P, NPT], F32)
            nc.vector.tensor_tensor(out=wB1[:], in0=wy[:], in1=wx[:], op=Alu.mult)

            # quad index: idx = (clamp(y0,-2,128)+2)*NQ + clamp(x0,-2,128)+2
            cy = cpool.tile([P, NPT], F32)
            nc.vector.tensor_scalar(out=cy[:], in0=y0[:], scalar1=-2.0, scalar2=128.0,
                                    op0=Alu.max, op1=Alu.min)
            cx = cpool.tile([P, NPT], F32)
            nc.vector.tensor_scalar(out=cx[:], in0=x0[:], scalar1=-2.0, scalar2=128.0,
                                    op0=Alu.max, op1=Alu.min)
            aff = cpool.tile([P, NPT], F32)
            nc.vector.tensor_scalar(out=aff[:], in0=cy[:], scalar1=float(NQ),
                                    scalar2=float(2 * NQ + 2), op0=Alu.mult,
                                    op1=Alu.add)
            nc.vector.tensor_tensor(out=aff[:], in0=aff[:], in1=cx[:], op=Alu.add)

            idx = cpool.tile([P, NPT], I32)
            nc.vector.tensor_copy(out=idx[:], in_=aff[:])

            import concourse.bass as _b

            sg = store_group
            assert NPT % sg == 0
            for rep in range(repeats):
                for g_i in range(NPT // sg):
                    afat = apool.tile([P, sg * C], BF16, tag="afat")
                    for k in range(sg):
                        t = g_i * sg + k
                        gq = wpool.tile([P, 4 * C], BF16, tag="gq")
                        nc.gpsimd.indirect_dma_start(
                            out=gq[:], out_offset=None, in_=qmap[:],
                            in_offset=_b.IndirectOffsetOnAxis(
                                ap=idx[:, t:t + 1], axis=0))
                        acc = wpool.tile([P, C], F32, tag="acc")
                        nc.vector.tensor_scalar(
                            out=acc[:], in0=gq[:, 0:C], scalar1=wA0[:, t:t + 1],
                            scalar2=None, op0=Alu.mult)
                        nc.vector.scalar_tensor_tensor(
                            out=acc[:], in0=gq[:, C:2 * C],
                            scalar=wA1[:, t:t + 1], in1=acc[:],
                            op0=Alu.mult, op1=Alu.add)
                        nc.vector.scalar_tensor_tensor(
                            out=acc[:], in0=gq[:, 2 * C:3 * C],
                            scalar=wB0[:, t:t + 1], in1=acc[:],
                            op0=Alu.mult, op1=Alu.add)
                        nc.vector.scalar_tensor_tensor(
                            out=afat[:, k * C:(k + 1) * C], in0=gq[:, 3 * C:4 * C],
                            scalar=wB1[:, t:t + 1], in1=acc[:],
                            op0=Alu.mult, op1=Alu.add)
                    nc.sync.dma_start(
                        out=out[:, g_i * sg * C:(g_i + 1) * sg * C],
                        in_=afat[:])

    nc.compile()
    return nc


def _grid_const():
    base = np.linspace(-1.0, 1.0, 7).astype(np.float32)
    by = np.repeat(base, 7)
    bxx = np.tile(base, 7)
    g = np.concatenate([by, bxx])[None, :]
    return np.ascontiguousarray(np.broadcast_to(g, (P, 2 * NPT)).astype(np.float32))


def _quad_features(features):
    f = np.zeros((HP, WP, C), dtype=np.float32)
    f[2:130, 2:130] = features
    fb = f.astype(ml_dtypes.bfloat16)
    q = np.concatenate(
        [fb[0:NQ, 0:NQ], fb[0:NQ, 1:NQ + 1], fb[1:NQ + 1, 0:NQ],
         fb[1:NQ + 1, 1:NQ + 1]], axis=2)
    return np.ascontiguousarray(q).reshape(NROW, 4 * C)


def _core_inputs(qmap, boxes, k):
    if "grid" not in _STATE:
        _STATE["grid"] = _grid_const()
    return {
        "qmap": qmap,
        "boxes": np.ascontiguousarray(boxes[k * P:(k + 1) * P]),
        "grid": _STATE["grid"],
    }


def kernel(features, boxes, image_height=128, image_width=128):
    from concourse.bass_utils import run_bass_kernel_spmd

    if "nc" not in _STATE:
        _STATE["nc"] = _build_nc()
    nc = _STATE["nc"]

    qmap = _quad_features(np.asarray(features, dtype=np.float32))
    boxes = np.asarray(boxes, dtype=np.float32)
    in_maps = [_core_inputs(qmap, boxes, k) for k in range(NCORES)]
    res = run_bass_kernel_spmd(
        nc, in_maps, core_ids=list(range(NCORES)),
        trace=_STATE.get("trace", False),
    )
    _STATE["last"] = res
    out = np.concatenate(
        [res.results[k]["out"].astype(np.float32).reshape(P, 7, 7, C)
         for k in range(NCORES)], axis=0
    )
    return out


# revision 4
# speedup vs baseline: 2.4737x; 1.5276x over previous
"""Bilinear RoI pooling (7x7) on 8 Trainium2 NeuronCores.

Strategy (data-parallel over RoIs, per the sharding hint):
  - B=1024 boxes split into 8 slices of 128; the feature map is replicated.
  - Host builds a "quad layout" map: Q[iy, ix] = the 2x2 corner block
    [F[y,x], F[y,x+1], F[y+1,x], F[y+1,x+1]] stored contiguously (4*C
    values), over a zero-padded canvas (2 pad rows/cols on every side).
    One indirect DMA descriptor per (box, grid-point) fetches all 4
    bilinear corners.
  - feat="int8": the map is linearly quantized (global scale s = max|f|/127)
    and the SWDGE gather casts int8 -> bf16 in-flight, halving HBM gather
    traffic; s is folded into the blend weights on device.
  - Corner indices are clamped to [-2, 128] so every out-of-bounds corner
    lands in an all-zero quad (the reference zeroes OOB contributions).
  - blend="pe": the 4-corner weighted sum runs on the tensor engine as 4
    accumulating matmuls whose stationary operand is diag(w_j) (built per
    point by eye*w on the vector engine); PSUM accumulates in f32 and the
    scalar engine copies PSUM -> bf16 SBUF.  The vector engine only does
    index/weight math; stores are grouped and the host upcasts to f32.

Device layout: partition = box (128/core); 49 grid points along free dim.
"""

import numpy as np
import ml_dtypes

P = 128          # boxes per core == SBUF partitions
C = 512          # channels
NPT = 49         # 7*7 grid points
HP = WP = 132    # padded canvas (2 zero rows/cols each side)
NQ = 131         # quad map is NQ x NQ cells of 4*C values
NROW = NQ * NQ
NCORES = 8
MAGIC = 12582912.0  # 1.5*2^23: x+MAGIC stays in [2^23,2^24) where ulp == 1

FEAT = "int8"    # default build config (test.py benches the same config)
BLEND = "pe"

_STATE = {}


# NOTE: multi-index offset APs pass CoreSim but produce garbage on real
# hardware — the HW indirect DMA only honors [P,1] offsets.
def _build_nc(repeats=1, feat=None, blend=None, bufs=8, abufs=3, dbufs=12,
              pbufs=6, store_group=7):
    import concourse.bass as bass
    import concourse.bacc as bacc
    import concourse.tile as tile
    from concourse import mybir

    feat = feat or FEAT
    blend = blend or BLEND

    F32 = mybir.dt.float32
    BF16 = mybir.dt.bfloat16
    I8 = mybir.dt.int8
    I32 = mybir.dt.int32
    Alu = mybir.AluOpType

    nc = bacc.Bacc()
    qdt = I8 if feat == "int8" else BF16
    qmap = nc.declare_dram_parameter("qmap", [NROW, 4 * C], qdt, isOutput=False)
    boxes = nc.declare_dram_parameter("boxes", [P, 4], F32, isOutput=False)
    grid = nc.declare_dram_parameter("grid", [P, 2 * NPT], F32, isOutput=False)
    if feat == "int8":
        scl = nc.declare_dram_parameter("scl", [P, 1], F32, isOutput=False)
    if blend == "pe":
        eye = nc.declare_dram_parameter("eye", [P, P], BF16, isOutput=False)
    out = nc.declare_dram_parameter("out", [P, NPT * C], BF16, isOutput=True)

    with tile.TileContext(nc) as tc:
        with (
            tc.tile_pool(name="const", bufs=1) as cpool,
            tc.tile_pool(name="apool", bufs=abufs) as apool,
            tc.tile_pool(name="dpool", bufs=dbufs) as dpool,
            tc.tile_pool(name="work", bufs=bufs) as wpool,
            tc.tile_pool(name="psum", bufs=pbufs, space="PSUM") as ppool,
        ):
            bx = cpool.tile([P, 4], F32)
            nc.sync.dma_start(out=bx[:], in_=boxes[:])
            g = cpool.tile([P, 2 * NPT], F32)
            nc.sync.dma_start(out=g[:], in_=grid[:])
            if feat == "int8":
                sc = cpool.tile([P, 1], F32)
                nc.sync.dma_start(out=sc[:], in_=scl[:])
            if blend == "pe":
                eye_t = cpool.tile([P, P], BF16)
                nc.sync.dma_start(out=eye_t[:], in_=eye[:])
            BY = g[:, 0:NPT]
            BX = g[:, NPT:2 * NPT]

            xc, yc = bx[:, 0:1], bx[:, 1:2]
            bw, bh = bx[:, 2:3], bx[:, 3:4]

            # per-box scale/translate: yf = BY*(0.5*bh-0.5) + (yc-1)
            sy = cpool.tile([P, 1], F32)
            nc.vector.tensor_scalar(out=sy[:], in0=bh, scalar1=0.5, scalar2=-0.5,
                                    op0=Alu.mult, op1=Alu.add)
            sx = cpool.tile([P, 1], F32)
            nc.vector.tensor_scalar(out=sx[:], in0=bw, scalar1=0.5, scalar2=-0.5,
                                    op0=Alu.mult, op1=Alu.add)
            ty = cpool.tile([P, 1], F32)
            nc.vector.tensor_scalar(out=ty[:], in0=yc, scalar1=-1.0, scalar2=None,
                                    op0=Alu.add)
            tx = cpool.tile([P, 1], F32)
            nc.vector.tensor_scalar(out=tx[:], in0=xc, scalar1=-1.0, scalar2=None,
                                    op0=Alu.add)

            yf = cpool.tile([P, NPT], F32)
            nc.vector.tensor_scalar(out=yf[:], in0=BY, scalar1=sy[:, 0:1],
                                    scalar2=ty[:, 0:1], op0=Alu.mult, op1=Alu.add)
            xf = cpool.tile([P, NPT], F32)
            nc.vector.tensor_scalar(out=xf[:], in0=BX, scalar1=sx[:, 0:1],
                                    scalar2=tx[:, 0:1], op0=Alu.mult, op1=Alu.add)

            def floor_frac(src, nm):
                r = cpool.tile([P, NPT], F32, tag=f"r{nm}")
                nc.vector.tensor_scalar(out=r[:], in0=src[:], scalar1=MAGIC,
                                        scalar2=-MAGIC, op0=Alu.add, op1=Alu.add)
                m = cpool.tile([P, NPT], F32, tag=f"m{nm}")
                nc.vector.tensor_tensor(out=m[:], in0=r[:], in1=src[:], op=Alu.is_gt)
                fl = cpool.tile([P, NPT], F32, tag=f"f{nm}")
                nc.vector.tensor_tensor(out=fl[:], in0=r[:], in1=m[:], op=Alu.subtract)
                fr = cpool.tile([P, NPT], F32, tag=f"w{nm}")
                nc.vector.tensor_tensor(out=fr[:], in0=src[:], in1=fl[:], op=Alu.subtract)
                return fl, fr

            y0, wy = floor_frac(yf[:], "y")
            x0, wx = floor_frac(xf[:], "x")

            wyc = cpool.tile([P, NPT], F32)
            nc.vector.tensor_scalar(out=wyc[:], in0=wy[:], scalar1=-1.0, scalar2=1.0,
                                    op0=Alu.mult, op1=Alu.add)
            wxc = cpool.tile([P, NPT], F32)
            nc.vector.tensor_scalar(out=wxc[:], in0=wx[:], scalar1=-1.0, scalar2=1.0,
                                    op0=Alu.mult, op1=Alu.add)

            wA0 = cpool.tile([P, NPT], F32)
            nc.vector.tensor_tensor(out=wA0[:], in0=wyc[:], in1=wxc[:], op=Alu.mult)
            wA1 = cpool.tile([P, NPT], F32)
            nc.vector.tensor_tensor(out=wA1[:], in0=wyc[:], in1=wx[:], op=Alu.mult)
            wB0 = cpool.tile([P, NPT], F32)
            nc.vector.tensor_tensor(out=wB0[:], in0=wy[:], in1=wxc[:], op=Alu.mult)
            wB1 = cpool.tile([P, NPT], F32)
            nc.vector.tensor_tensor(out=wB1[:], in0=wy[:], in1=wx[:], op=Alu.mult)

            wts = [wA0, wA1, wB0, wB1]
            if feat == "int8":
                # fold the dequant scale into the blend weights
                for wt in wts:
                    nc.vector.tensor_scalar(out=wt[:], in0=wt[:],
                                            scalar1=sc[:, 0:1], scalar2=None,
                                            op0=Alu.mult)

            # quad index: idx = (clamp(y0,-2,128)+2)*NQ + clamp(x0,-2,128)+2
            cy = cpool.tile([P, NPT], F32)
            nc.vector.tensor_scalar(out=cy[:], in0=y0[:], scalar1=-2.0, scalar2=128.0,
                                    op0=Alu.max, op1=Alu.min)
            cx = cpool.tile([P, NPT], F32)
            nc.vector.tensor_scalar(out=cx[:], in0=x0[:], scalar1=-2.0, scalar2=128.0,
                                    op0=Alu.max, op1=Alu.min)
            aff = cpool.tile([P, NPT], F32)
            nc.vector.tensor_scalar(out=aff[:], in0=cy[:], scalar1=float(NQ),
                                    scalar2=float(2 * NQ + 2), op0=Alu.mult,
                                    op1=Alu.add)
            nc.vector.tensor_tensor(out=aff[:], in0=aff[:], in1=cx[:], op=Alu.add)

            idx = cpool.tile([P, NPT], I32)
            nc.vector.tensor_copy(out=idx[:], in_=aff[:])

            import concourse.bass as _b

            sg = store_group
            assert NPT % sg == 0
            for rep in range(repeats):
                for g_i in range(NPT // sg):
                    afat = apool.tile([P, sg * C], BF16, tag="afat")
                    for k in range(sg):
                        t = g_i * sg + k
                        gq = wpool.tile([P, 4 * C], BF16, tag="gq")
                        nc.gpsimd.indirect_dma_start(
                            out=gq[:], out_offset=None, in_=qmap[:],
                            in_offset=_b.IndirectOffsetOnAxis(
                                ap=idx[:, t:t + 1], axis=0))
                        if blend == "pe":
                            dg = dpool.tile([P, 4, P], BF16, tag="dg")
                            for j in range(4):
                                nc.vector.tensor_scalar(
                                    out=dg[:, j], in0=eye_t[:],
                                    scalar1=wts[j][:, t:t + 1], scalar2=None,
                                    op0=Alu.mult)
                            ps = ppool.tile([P, C], mybir.dt.float32, tag="ps")
                            for j in range(4):
                                nc.tensor.matmul(
                                    ps[:], dg[:, j], gq[:, j * C:(j + 1) * C],
                                    start=(j == 0), stop=(j == 3))
                            nc.scalar.copy(out=afat[:, k * C:(k + 1) * C],
                                           in_=ps[:])
                        else:
                            acc = wpool.tile([P, C], F32, tag="acc")
                            nc.vector.tensor_scalar(
                                out=acc[:], in0=gq[:, 0:C],
                                scalar1=wA0[:, t:t + 1],
                                scalar2=None, op0=Alu.mult)
                            nc.vector.scalar_tensor_tensor(
                                out=acc[:], in0=gq[:, C:2 * C],
                                scalar=wA1[:, t:t + 1], in1=acc[:],
                                op0=Alu.mult, op1=Alu.add)
                            nc.vector.scalar_tensor_tensor(
                                out=acc[:], in0=gq[:, 2 * C:3 * C],
                                scalar=wB0[:, t:t + 1], in1=acc[:],
                                op0=Alu.mult, op1=Alu.add)
                            nc.vector.scalar_tensor_tensor(
                                out=afat[:, k * C:(k + 1) * C],
                                in0=gq[:, 3 * C:4 * C],
                                scalar=wB1[:, t:t + 1], in1=acc[:],
                                op0=Alu.mult, op1=Alu.add)
                    nc.sync.dma_start(
                        out=out[:, g_i * sg * C:(g_i + 1) * sg * C],
                        in_=afat[:])

    nc.compile()
    return nc


def _grid_const():
    base = np.linspace(-1.0, 1.0, 7).astype(np.float32)
    by = np.repeat(base, 7)
    bxx = np.tile(base, 7)
    g = np.concatenate([by, bxx])[None, :]
    return np.ascontiguousarray(np.broadcast_to(g, (P, 2 * NPT)).astype(np.float32))


def _quad_features(features, feat=None):
    feat = feat or FEAT
    f = np.zeros((HP, WP, C), dtype=np.float32)
    f[2:130, 2:130] = features
    if feat == "int8":
        s = float(np.max(np.abs(features))) / 127.0
        if s == 0.0:
            s = 1.0
        fb = np.clip(np.rint(f / s), -127, 127).astype(np.int8)
    else:
        s = None
        fb = f.astype(ml_dtypes.bfloat16)
    q = np.concatenate(
        [fb[0:NQ, 0:NQ], fb[0:NQ, 1:NQ + 1], fb[1:NQ + 1, 0:NQ],
         fb[1:NQ + 1, 1:NQ + 1]], axis=2)
    return np.ascontiguousarray(q).reshape(NROW, 4 * C), s


def _core_inputs(qmap, boxes, k, s=None):
    if "grid" not in _STATE:
        _STATE["grid"] = _grid_const()
        _STATE["eye"] = np.eye(P, dtype=ml_dtypes.bfloat16)
    m = {
        "qmap": qmap,
        "boxes": np.ascontiguousarray(boxes[k * P:(k + 1) * P]),
        "grid": _STATE["grid"],
    }
    if BLEND == "pe":
        m["eye"] = _STATE["eye"]
    if s is not None:
        m["scl"] = np.full((P, 1), s, dtype=np.float32)
    return m


def kernel(features, boxes, image_height=128, image_width=128):
    from concourse.bass_utils import run_bass_kernel_spmd

    if "nc" not in _STATE:
        _STATE["nc"] = _build_nc()
    nc = _STATE["nc"]

    qmap, s = _quad_features(np.asarray(features, dtype=np.float32))
    boxes = np.asarray(boxes, dtype=np.float32)
    in_maps = [_core_inputs(qmap, boxes, k, s) for k in range(NCORES)]
    res = run_bass_kernel_spmd(
        nc, in_maps, core_ids=list(range(NCORES)),
        trace=_STATE.get("trace", False),
    )
    _STATE["last"] = res
    out = np.concatenate(
        [res.results[k]["out"].astype(np.float32).reshape(P, 7, 7, C)
         for k in range(NCORES)], axis=0
    )
    return out


# revision 7
# speedup vs baseline: 4.0763x; 1.6479x over previous
"""Bilinear RoI pooling (7x7) on 8 Trainium2 NeuronCores.

Strategy (data-parallel over RoIs, per the sharding hint):
  - B=1024 boxes split into 8 slices of 128; the feature map is replicated.
  - Host builds a "quad layout" map: Q[iy, ix] = the 2x2 corner block
    [F[y,x], F[y,x+1], F[y+1,x], F[y+1,x+1]] stored contiguously (4*C
    values), over a zero-padded canvas (2 pad rows/cols on every side).
    One indirect DMA descriptor per (box, grid-point) fetches all 4
    bilinear corners.
  - feat="int8": the map is linearly quantized (global scale s = max|f|/127)
    and the SWDGE gather casts int8 -> bf16 in-flight, halving HBM gather
    traffic; s is folded into the blend weights on device.
  - Corner indices are clamped to [-2, 128] so every out-of-bounds corner
    lands in an all-zero quad (the reference zeroes OOB contributions).
  - blend="pe": the 4-corner weighted sum runs on the tensor engine as 4
    accumulating matmuls whose stationary operand is diag(w_j) (built per
    point by eye*w on the vector engine); PSUM accumulates in f32 and the
    scalar engine copies PSUM -> bf16 SBUF.  The vector engine only does
    index/weight math; stores are grouped and the host upcasts to f32.

Device layout: partition = box (128/core); 49 grid points along free dim.
"""

import numpy as np
import ml_dtypes

P = 128          # boxes per core == SBUF partitions
C = 512          # channels
NPT = 49         # 7*7 grid points
HP = WP = 132    # padded canvas (2 zero rows/cols each side)
NQ = 131         # quad map is NQ x NQ cells of 4*C values
NROW = NQ * NQ
NCORES = 8
MAGIC = 12582912.0  # 1.5*2^23: x+MAGIC stays in [2^23,2^24) where ulp == 1

FEAT = "int8"    # default build config (test.py benches the same config)
BLEND = "pe"
OSTORE = "int8"  # output store dtype; int8 is dequantized host-side by the same s

_STATE = {}


# NOTE: multi-index offset APs pass CoreSim but produce garbage on real
# hardware — the HW indirect DMA only honors [P,1] offsets.
def _build_nc(repeats=1, feat=None, blend=None, ostore=None, bufs=8, abufs=3,
              dbufs=12, pbufs=6, store_group=7):
    import concourse.bass as bass
    import concourse.bacc as bacc
    import concourse.tile as tile
    from concourse import mybir

    feat = feat or FEAT
    blend = blend or BLEND
    ostore = ostore or OSTORE
    assert ostore == "bf16" or (feat == "int8" and blend == "pe")

    F32 = mybir.dt.float32
    BF16 = mybir.dt.bfloat16
    I8 = mybir.dt.int8
    I32 = mybir.dt.int32
    Alu = mybir.AluOpType

    nc = bacc.Bacc()
    qdt = I8 if feat == "int8" else BF16
    qmap = nc.declare_dram_parameter("qmap", [NROW, 4 * C], qdt, isOutput=False)
    boxes = nc.declare_dram_parameter("boxes", [P, 4], F32, isOutput=False)
    grid = nc.declare_dram_parameter("grid", [P, 2 * NPT], F32, isOutput=False)
    if feat == "int8" and ostore == "bf16":
        scl = nc.declare_dram_parameter("scl", [P, 1], F32, isOutput=False)
    if blend == "pe":
        eye = nc.declare_dram_parameter("eye", [P, P], BF16, isOutput=False)
    odt = I8 if ostore == "int8" else BF16
    out = nc.declare_dram_parameter("out", [P, NPT * C], odt, isOutput=True)

    with tile.TileContext(nc) as tc:
        with (
            tc.tile_pool(name="const", bufs=1) as cpool,
            tc.tile_pool(name="apool", bufs=abufs) as apool,
            tc.tile_pool(name="dpool", bufs=dbufs) as dpool,
            tc.tile_pool(name="work", bufs=bufs) as wpool,
            tc.tile_pool(name="psum", bufs=pbufs, space="PSUM") as ppool,
        ):
            bx = cpool.tile([P, 4], F32)
            nc.sync.dma_start(out=bx[:], in_=boxes[:])
            g = cpool.tile([P, 2 * NPT], F32)
            nc.sync.dma_start(out=g[:], in_=grid[:])
            if feat == "int8" and ostore == "bf16":
                sc = cpool.tile([P, 1], F32)
                nc.sync.dma_start(out=sc[:], in_=scl[:])
            if blend == "pe":
                eye_t = cpool.tile([P, P], BF16)
                nc.sync.dma_start(out=eye_t[:], in_=eye[:])
            BY = g[:, 0:NPT]
            BX = g[:, NPT:2 * NPT]

            xc, yc = bx[:, 0:1], bx[:, 1:2]
            bw, bh = bx[:, 2:3], bx[:, 3:4]

            # per-box scale/translate: yf = BY*(0.5*bh-0.5) + (yc-1)
            sy = cpool.tile([P, 1], F32)
            nc.vector.tensor_scalar(out=sy[:], in0=bh, scalar1=0.5, scalar2=-0.5,
                                    op0=Alu.mult, op1=Alu.add)
            sx = cpool.tile([P, 1], F32)
            nc.vector.tensor_scalar(out=sx[:], in0=bw, scalar1=0.5, scalar2=-0.5,
                                    op0=Alu.mult, op1=Alu.add)
            ty = cpool.tile([P, 1], F32)
            nc.vector.tensor_scalar(out=ty[:], in0=yc, scalar1=-1.0, scalar2=None,
                                    op0=Alu.add)
            tx = cpool.tile([P, 1], F32)
            nc.vector.tensor_scalar(out=tx[:], in0=xc, scalar1=-1.0, scalar2=None,
                                    op0=Alu.add)

            yf = cpool.tile([P, NPT], F32)
            nc.vector.tensor_scalar(out=yf[:], in0=BY, scalar1=sy[:, 0:1],
                                    scalar2=ty[:, 0:1], op0=Alu.mult, op1=Alu.add)
            xf = cpool.tile([P, NPT], F32)
            nc.vector.tensor_scalar(out=xf[:], in0=BX, scalar1=sx[:, 0:1],
                                    scalar2=tx[:, 0:1], op0=Alu.mult, op1=Alu.add)

            def floor_frac(src, nm):
                r = cpool.tile([P, NPT], F32, tag=f"r{nm}")
                nc.vector.tensor_scalar(out=r[:], in0=src[:], scalar1=MAGIC,
                                        scalar2=-MAGIC, op0=Alu.add, op1=Alu.add)
                m = cpool.tile([P, NPT], F32, tag=f"m{nm}")
                nc.vector.tensor_tensor(out=m[:], in0=r[:], in1=src[:], op=Alu.is_gt)
                fl = cpool.tile([P, NPT], F32, tag=f"f{nm}")
                nc.vector.tensor_tensor(out=fl[:], in0=r[:], in1=m[:], op=Alu.subtract)
                fr = cpool.tile([P, NPT], F32, tag=f"w{nm}")
                nc.vector.tensor_tensor(out=fr[:], in0=src[:], in1=fl[:], op=Alu.subtract)
                return fl, fr

            y0, wy = floor_frac(yf[:], "y")
            x0, wx = floor_frac(xf[:], "x")

            wyc = cpool.tile([P, NPT], F32)
            nc.vector.tensor_scalar(out=wyc[:], in0=wy[:], scalar1=-1.0, scalar2=1.0,
                                    op0=Alu.mult, op1=Alu.add)
            wxc = cpool.tile([P, NPT], F32)
            nc.vector.tensor_scalar(out=wxc[:], in0=wx[:], scalar1=-1.0, scalar2=1.0,
                                    op0=Alu.mult, op1=Alu.add)

            wA0 = cpool.tile([P, NPT], F32)
            nc.vector.tensor_tensor(out=wA0[:], in0=wyc[:], in1=wxc[:], op=Alu.mult)
            wA1 = cpool.tile([P, NPT], F32)
            nc.vector.tensor_tensor(out=wA1[:], in0=wyc[:], in1=wx[:], op=Alu.mult)
            wB0 = cpool.tile([P, NPT], F32)
            nc.vector.tensor_tensor(out=wB0[:], in0=wy[:], in1=wxc[:], op=Alu.mult)
            wB1 = cpool.tile([P, NPT], F32)
            nc.vector.tensor_tensor(out=wB1[:], in0=wy[:], in1=wx[:], op=Alu.mult)

            wts = [wA0, wA1, wB0, wB1]
            if feat == "int8" and ostore == "bf16":
                # fold the dequant scale into the blend weights
                for wt in wts:
                    nc.vector.tensor_scalar(out=wt[:], in0=wt[:],
                                            scalar1=sc[:, 0:1], scalar2=None,
                                            op0=Alu.mult)

            # quad index: idx = (clamp(y0,-2,128)+2)*NQ + clamp(x0,-2,128)+2
            cy = cpool.tile([P, NPT], F32)
            nc.vector.tensor_scalar(out=cy[:], in0=y0[:], scalar1=-2.0, scalar2=128.0,
                                    op0=Alu.max, op1=Alu.min)
            cx = cpool.tile([P, NPT], F32)
            nc.vector.tensor_scalar(out=cx[:], in0=x0[:], scalar1=-2.0, scalar2=128.0,
                                    op0=Alu.max, op1=Alu.min)
            aff = cpool.tile([P, NPT], F32)
            nc.vector.tensor_scalar(out=aff[:], in0=cy[:], scalar1=float(NQ),
                                    scalar2=float(2 * NQ + 2), op0=Alu.mult,
                                    op1=Alu.add)
            nc.vector.tensor_tensor(out=aff[:], in0=aff[:], in1=cx[:], op=Alu.add)

            idx = cpool.tile([P, NPT], I32)
            nc.vector.tensor_copy(out=idx[:], in_=aff[:])

            import concourse.bass as _b

            sg = store_group
            assert NPT % sg == 0
            for rep in range(repeats):
                for g_i in range(NPT // sg):
                    afat = apool.tile([P, sg * C], odt, tag="afat")
                    for k in range(sg):
                        t = g_i * sg + k
                        gq = wpool.tile([P, 4 * C], BF16, tag="gq")
                        nc.gpsimd.indirect_dma_start(
                            out=gq[:], out_offset=None, in_=qmap[:],
                            in_offset=_b.IndirectOffsetOnAxis(
                                ap=idx[:, t:t + 1], axis=0))
                        if blend == "pe":
                            dg = dpool.tile([P, 4, P], BF16, tag="dg")
                            for j in range(4):
                                nc.vector.tensor_scalar(
                                    out=dg[:, j], in0=eye_t[:],
                                    scalar1=wts[j][:, t:t + 1], scalar2=None,
                                    op0=Alu.mult)
                            ps = ppool.tile([P, C], mybir.dt.float32, tag="ps")
                            for j in range(4):
                                nc.tensor.matmul(
                                    ps[:], dg[:, j], gq[:, j * C:(j + 1) * C],
                                    start=(j == 0), stop=(j == 3))
                            nc.scalar.copy(out=afat[:, k * C:(k + 1) * C],
                                           in_=ps[:])
                        else:
                            acc = wpool.tile([P, C], F32, tag="acc")
                            nc.vector.tensor_scalar(
                                out=acc[:], in0=gq[:, 0:C],
                                scalar1=wA0[:, t:t + 1],
                                scalar2=None, op0=Alu.mult)
                            nc.vector.scalar_tensor_tensor(
                                out=acc[:], in0=gq[:, C:2 * C],
                                scalar=wA1[:, t:t + 1], in1=acc[:],
                                op0=Alu.mult, op1=Alu.add)
                            nc.vector.scalar_tensor_tensor(
                                out=acc[:], in0=gq[:, 2 * C:3 * C],
                                scalar=wB0[:, t:t + 1], in1=acc[:],
                                op0=Alu.mult, op1=Alu.add)
                            nc.vector.scalar_tensor_tensor(
                                out=afat[:, k * C:(k + 1) * C],
                                in0=gq[:, 3 * C:4 * C],
                                scalar=wB1[:, t:t + 1], in1=acc[:],
                                op0=Alu.mult, op1=Alu.add)
                    nc.sync.dma_start(
                        out=out[:, g_i * sg * C:(g_i + 1) * sg * C],
                        in_=afat[:])

    nc.compile()
    return nc


def _grid_const():
    base = np.linspace(-1.0, 1.0, 7).astype(np.float32)
    by = np.repeat(base, 7)
    bxx = np.tile(base, 7)
    g = np.concatenate([by, bxx])[None, :]
    return np.ascontiguousarray(np.broadcast_to(g, (P, 2 * NPT)).astype(np.float32))


def _quad_features(features, feat=None):
    feat = feat or FEAT
    f = np.zeros((HP, WP, C), dtype=np.float32)
    f[2:130, 2:130] = features
    if feat == "int8":
        s = float(np.max(np.abs(features))) / 127.0
        if s == 0.0:
            s = 1.0
        fb = np.clip(np.rint(f / s), -127, 127).astype(np.int8)
    else:
        s = None
        fb = f.astype(ml_dtypes.bfloat16)
    q = np.concatenate(
        [fb[0:NQ, 0:NQ], fb[0:NQ, 1:NQ + 1], fb[1:NQ + 1, 0:NQ],
         fb[1:NQ + 1, 1:NQ + 1]], axis=2)
    return np.ascontiguousarray(q).reshape(NROW, 4 * C), s


def _core_inputs(qmap, boxes, k, s=None):
    if "grid" not in _STATE:
        _STATE["grid"] = _grid_const()
        _STATE["eye"] = np.eye(P, dtype=ml_dtypes.bfloat16)
    m = {
        "qmap": qmap,
        "boxes": np.ascontiguousarray(boxes[k * P:(k + 1) * P]),
        "grid": _STATE["grid"],
    }
    if BLEND == "pe":
        m["eye"] = _STATE["eye"]
    if s is not None and OSTORE == "bf16":
        m["scl"] = np.full((P, 1), s, dtype=np.float32)
    return m


def kernel(features, boxes, image_height=128, image_width=128):
    from concourse.bass_utils import run_bass_kernel_spmd

    if "nc" not in _STATE:
        _STATE["nc"] = _build_nc()
    nc = _STATE["nc"]

    qmap, s = _quad_features(np.asarray(features, dtype=np.float32))
    boxes = np.asarray(boxes, dtype=np.float32)
    in_maps = [_core_inputs(qmap, boxes, k, s) for k in range(NCORES)]
    res = run_bass_kernel_spmd(
        nc, in_maps, core_ids=list(range(NCORES)),
        trace=_STATE.get("trace", False),
    )
    _STATE["last"] = res
    out = np.concatenate(
        [res.results[k]["out"].astype(np.float32).reshape(P, 7, 7, C)
         for k in range(NCORES)], axis=0
    )
    if OSTORE == "int8":
        out *= np.float32(s)  # dequantize: device stored round(sum(w*q))
    return out
